# revision 1
# baseline (speedup 1.0000x reference)
"""Trainium2 Bass kernel for nn_Connection_v5extend (8-core data-parallel).

kernel(**inputs) takes the FULL unsharded inputs (as produced by
setup_inputs) and returns the FULL [4096, 256] float32 output.

Strategy: shard the batch dim (4096) into 8 contiguous row blocks, one per
NeuronCore; replicate the tiny MLP weights.  The reference's [B, D, D]
Jacobian is never materialized — the output reduces to 6 small matmuls plus
elementwise work:

    x, v  = input[:, :D], input[:, D:]
    h     = x @ W1.T + b1;  a = relu(h);  mask = (h > 0)
    s     = sigmoid(a @ W2.T + b2);  sig = s*(1-s)
    u1    = (v^2 * sgn * sig) @ W2
    term1 = (u1 * mask) @ W1
    T2p   = (mask * (v @ W1.T)) @ W2.T
    dv    = (2/(s+C)) * (v*sig*T2p - 0.5*sgn*term1)
    out   = concat([v, dv], axis=1)

On-chip layout is feature-major (features on SBUF partitions, batch on the
free dim); x/v are PE-transposed on load and dv transposed back on store.
The smooth-path matmuls run in float32r (fp32 storage, reduced-precision
multiplies); the h matmul stays true fp32 so the relu mask matches the
reference.  The main chain is split into batch segments so the tensor /
scalar / vector engines pipeline against each other.
"""

import sys

sys.path.insert(0, "/opt/trn_rl_repo")

import numpy as np

import concourse.bass as bass  # noqa: F401
import concourse.bacc as bacc
import concourse.mybir as mybir
import concourse.tile as tile
from concourse.masks import make_identity
from concourse.bass_utils import run_bass_kernel_spmd

F32 = mybir.dt.float32
BF16 = mybir.dt.bfloat16
AF = mybir.ActivationFunctionType
ALU = mybir.AluOpType

D = 128
CONST = 0.618
SIGN = 4
N_CORES = 8
BATCH = 4096
B = BATCH // N_CORES  # rows per core


def _build(nc, B=512, mm_dtype="float32r", n_seg=2):
    """Emit the per-core kernel.  mm_dtype: dtype of the smooth-path matmuls
    ('float32' | 'float32r' | 'bfloat16'); the h matmul is always fp32.
    n_seg: batch segments for engine pipelining."""
    NCH = B // D           # 128-row chunks for PE transposes
    SEG = B // n_seg       # batch columns per pipeline segment
    assert NCH * D == B and SEG * n_seg == B and SEG % D == 0

    mdt = {"float32": F32, "bfloat16": BF16,
           "float32r": mybir.dt.float32r}[mm_dtype]

    inp = nc.dram_tensor("inp", [B, 2 * D], F32, kind="ExternalInput").ap()
    W1 = nc.dram_tensor("W1", [2 * D, D], F32, kind="ExternalInput").ap()
    b1 = nc.dram_tensor("b1", [2 * D], F32, kind="ExternalInput").ap()
    W2 = nc.dram_tensor("W2", [D, 2 * D], F32, kind="ExternalInput").ap()
    b2 = nc.dram_tensor("b2", [D], F32, kind="ExternalInput").ap()
    out = nc.dram_tensor("out", [B, 2 * D], F32, kind="ExternalOutput").ap()

    with tile.TileContext(nc) as tc:
        with (
            tc.tile_pool(name="consts", bufs=1) as consts,
            tc.tile_pool(name="sb", bufs=1) as sb,
            tc.tile_pool(name="seg", bufs=2) as sgp,
            tc.tile_pool(name="ps", bufs=2, space="PSUM") as ps,
            tc.tile_pool(name="pst", bufs=2, space="PSUM") as pst,
            tc.tile_pool(name="psio", bufs=1, space="PSUM") as psio,
        ):
            # ---------------- constants ----------------
            # Dummy sigmoid first so walrus picks the sigmoid ACT table set
            # once (it contains relu/square/identity/copy too).
            warm = consts.tile([D, 1], F32, tag="warm")
            nc.vector.memset(warm[:], 0.0)
            nc.scalar.activation(warm[:], warm[:], AF.Sigmoid)

            ident = consts.tile([D, D], F32, tag="ident")
            make_identity(nc, ident[:])

            b1c = consts.tile([D, 2], F32, tag="b1c")
            nc.gpsimd.dma_start(b1c[:], b1.rearrange("(h p) -> p h", p=D))
            b2c = consts.tile([D, 1], F32, tag="b2c")
            nc.gpsimd.dma_start(b2c[:], b2.rearrange("(p o) -> p o", o=1))

            sgnc = consts.tile([D, 1], F32, tag="sgnc")
            nc.vector.memset(sgnc[:], 1.0)
            nc.vector.memset(sgnc[:SIGN, :], -1.0)
            nhsgn = consts.tile([D, 1], F32, tag="nhsgn")
            nc.vector.memset(nhsgn[:], -0.5)
            nc.vector.memset(nhsgn[:SIGN, :], 0.5)
            c309 = consts.tile([D, 1], F32, tag="c309")
            nc.vector.memset(c309[:], CONST / 2.0)

            # ---------------- weights ----------------
            w1kf32 = consts.tile([D, 2, D], F32, tag="w1kf32")
            nc.gpsimd.dma_start(w1kf32[:], W1.rearrange("(h p) j -> p h j", p=D))
            if mdt == F32:
                w1k = w1kf32
            else:
                w1k = consts.tile([D, 2, D], mdt, tag="w1k")
                nc.vector.tensor_copy(w1k[:], w1kf32[:])

            w1T = consts.tile([D, 2 * D], F32, tag="w1T")
            for h in range(2):
                tp = pst.tile([D, D], F32, tag="tp")
                nc.tensor.transpose(tp[:], w1kf32[:, h, :], ident[:])
                nc.scalar.copy(w1T[:, h * D:(h + 1) * D], tp[:])
            if mdt == F32:
                w1Tm = w1T
            else:
                w1Tm = consts.tile([D, 2 * D], mdt, tag="w1Tm")
                nc.vector.tensor_copy(w1Tm[:], w1T[:])

            w2nf32 = consts.tile([D, 2 * D], F32, tag="w2nf32")
            nc.gpsimd.dma_start(w2nf32[:], W2)
            if mdt == F32:
                w2n = w2nf32
            else:
                w2n = consts.tile([D, 2 * D], mdt, tag="w2n")
                nc.vector.tensor_copy(w2n[:], w2nf32[:])

            w2T = consts.tile([D, 2, D], mdt, tag="w2T")
            for h in range(2):
                tp = pst.tile([D, D], F32, tag="tp")
                nc.tensor.transpose(tp[:], w2nf32[:, h * D:(h + 1) * D], ident[:])
                if mdt == F32:
                    nc.scalar.copy(w2T[:, h, :], tp[:])
                else:
                    nc.vector.tensor_copy(w2T[:, h, :], tp[:])

            # ---------------- input: chunked DMAs on both HWDGE rings -------
            # per-chunk loads overlap the PE transposes and the first segment
            it = sb.tile([D, NCH, 2 * D], F32, tag="in_t")
            for c in range(NCH):
                eng = nc.sync if c % 2 == 0 else nc.scalar
                eng.dma_start(it[:, c, :], inp[c * D:(c + 1) * D, :])
            # v passthrough (SWDGE queue, off the critical path)
            nc.gpsimd.dma_start(
                out.rearrange("(c p) f -> p c f", p=D)[:, :, 0:D],
                it[:, :, D:2 * D])

            # xT fp32 (exact, feeds the fp32 h matmul); vT stored as mdt
            # (f32r rounding is ~1e-5 relative — harmless in the smooth path)
            xTt = sb.tile([D, B], F32, tag="xTt")
            vdt = F32 if mdt == BF16 else mdt
            vTt = sb.tile([D, B], vdt, tag="vTt")
            itp = psio.tile([D, NCH, 2, D], F32, tag="iotp")
            for c in range(NCH):
                nc.tensor.transpose(itp[:, c, 0, :], it[:, c, 0:D], ident[:])
                nc.tensor.transpose(itp[:, c, 1, :], it[:, c, D:2 * D], ident[:])
                nc.vector.tensor_copy(xTt[:, c * D:(c + 1) * D], itp[:, c, 0, :])
                nc.vector.tensor_copy(vTt[:, c * D:(c + 1) * D], itp[:, c, 1, :])
            xT = xTt[:]
            vT = vTt[:].bitcast(F32)
            if mdt == BF16:
                vTmt = sb.tile([D, B], mdt, tag="vTmt")
                nc.vector.tensor_copy(vTmt[:], vTt[:])
                vTm = vTmt[:]
            else:
                vTm = vTt[:]

            # ---------------- main chain, pipelined over segments ----------
            dvT = sb.tile([D, B], F32, tag="dvT")
            for g in range(n_seg):
                sl = slice(g * SEG, (g + 1) * SEG)
                xTg, vTg = xT[:, sl], vT[:, sl]
                vTmg = vTm[:, sl]

                hps = ps.tile([D, 2, SEG], F32, tag="psbig")
                nc.tensor.matmul(hps[:, 0, :], w1T[:, 0:D], xTg)
                nc.tensor.matmul(hps[:, 1, :], w1T[:, D:2 * D], xTg)

                a = sgp.tile([D, 2, SEG], mdt, tag="a")
                for h in range(2):
                    nc.scalar.activation(a[:, h, :], hps[:, h, :], AF.Relu,
                                         bias=b1c[:, h:h + 1])
                mask = sgp.tile([D, 2, SEG], mdt, tag="mask")
                nc.vector.tensor_single_scalar(
                    mask.rearrange("p s b -> p (s b)"),
                    a.rearrange("p s b -> p (s b)"), 0.0, ALU.is_gt)

                zps = ps.tile([D, SEG], F32, tag="pssmall")
                nc.tensor.matmul(zps[:], w2T[:, 0, :], a[:, 0, :],
                                 start=True, stop=False)
                nc.tensor.matmul(zps[:], w2T[:, 1, :], a[:, 1, :],
                                 start=False, stop=True)

                s = sgp.tile([D, SEG], F32, tag="s")
                nc.scalar.activation(s[:], zps[:], AF.Sigmoid, bias=b2c[:, 0:1])
                ssq = sgp.tile([D, SEG], F32, tag="ssq")
                nc.scalar.square(ssq[:], s[:])
                sig = sgp.tile([D, SEG], F32, tag="sig")
                nc.vector.tensor_sub(sig[:], s[:], ssq[:])

                vsq = sgp.tile([D, SEG], F32, tag="vsq")
                nc.scalar.square(vsq[:], vTg)
                p = sgp.tile([D, SEG], mdt, tag="p")
                nc.vector.scalar_tensor_tensor(p[:], vsq[:], sgnc[:, 0:1],
                                               sig[:], ALU.mult, ALU.mult)

                u1ps = ps.tile([D, 2, SEG], F32, tag="psbig")
                nc.tensor.matmul(u1ps[:, 0, :], w2n[:, 0:D], p[:])
                nc.tensor.matmul(u1ps[:, 1, :], w2n[:, D:2 * D], p[:])

                r = sgp.tile([D, 2, SEG], mdt, tag="r")
                nc.vector.tensor_mul(r.rearrange("p s b -> p (s b)"),
                                     u1ps.rearrange("p s b -> p (s b)"),
                                     mask.rearrange("p s b -> p (s b)"))

                t1ps = ps.tile([D, SEG], F32, tag="pssmall")
                nc.tensor.matmul(t1ps[:], w1k[:, 0, :], r[:, 0, :],
                                 start=True, stop=False)
                nc.tensor.matmul(t1ps[:], w1k[:, 1, :], r[:, 1, :],
                                 start=False, stop=True)

                wps = ps.tile([D, 2, SEG], F32, tag="psbig")
                nc.tensor.matmul(wps[:, 0, :], w1Tm[:, 0:D], vTmg)
                nc.tensor.matmul(wps[:, 1, :], w1Tm[:, D:2 * D], vTmg)

                mw = sgp.tile([D, 2, SEG], mdt, tag="mw")
                nc.vector.tensor_mul(mw.rearrange("p s b -> p (s b)"),
                                     wps.rearrange("p s b -> p (s b)"),
                                     mask.rearrange("p s b -> p (s b)"))

                t2ps = ps.tile([D, SEG], F32, tag="pssmall")
                nc.tensor.matmul(t2ps[:], w2T[:, 0, :], mw[:, 0, :],
                                 start=True, stop=False)
                nc.tensor.matmul(t2ps[:], w2T[:, 1, :], mw[:, 1, :],
                                 start=False, stop=True)

                # dv = r02 * (v*sig*T2p - 0.5*sgn*term1)
                vsig = sgp.tile([D, SEG], F32, tag="vsig")
                nc.gpsimd.tensor_mul(vsig[:], vTg, sig[:])
                a2 = sgp.tile([D, SEG], F32, tag="a2")
                nc.vector.tensor_mul(a2[:], vsig[:], t2ps[:])
                cc = sgp.tile([D, SEG], F32, tag="cc")
                nc.vector.scalar_tensor_tensor(cc[:], t1ps[:], nhsgn[:, 0:1],
                                               a2[:], ALU.mult, ALU.add)

                r02p = sgp.tile([D, SEG], F32, tag="r02p")
                nc.scalar.activation(r02p[:], s[:], AF.Identity,
                                     bias=c309[:, 0:1], scale=0.5)
                r02 = sgp.tile([D, SEG], F32, tag="r02")
                nc.vector.reciprocal_approx_fast(r02[:], r02p[:])

                nc.vector.tensor_mul(dvT[:, sl], cc[:], r02[:])

            # ---------------- output: transpose, one copy, one DMA ---------
            otp = psio.tile([D, NCH, D], F32, tag="iotp")
            for c in range(NCH):
                nc.tensor.transpose(otp[:, c, :], dvT[:, c * D:(c + 1) * D],
                                    ident[:])
            ot = sb.tile([D, NCH, D], F32, tag="out_t")
            nc.scalar.copy(ot[:], otp[:])
            nc.sync.dma_start(
                out.rearrange("(c p) f -> p c f", p=D)[:, :, D:2 * D], ot[:])

    return nc


_CACHE = {}


def _get_nc(mm_dtype="float32r"):
    key = mm_dtype
    if key not in _CACHE:
        nc = bacc.Bacc("TRN2", target_bir_lowering=False, debug=False,
                       num_devices=N_CORES)
        _build(nc, B=B, mm_dtype=mm_dtype)
        nc.compile()
        _CACHE[key] = nc
    return _CACHE[key]


def kernel(t, input_, W1, b1, W2, b2):
    input_ = np.ascontiguousarray(np.asarray(input_, dtype=np.float32))
    W1 = np.ascontiguousarray(np.asarray(W1, dtype=np.float32))
    b1 = np.ascontiguousarray(np.asarray(b1, dtype=np.float32))
    W2 = np.ascontiguousarray(np.asarray(W2, dtype=np.float32))
    b2 = np.ascontiguousarray(np.asarray(b2, dtype=np.float32))
    assert input_.shape == (BATCH, 2 * D)

    nc = _get_nc()
    in_maps = [
        {"inp": input_[c * B:(c + 1) * B], "W1": W1, "b1": b1, "W2": W2, "b2": b2}
        for c in range(N_CORES)
    ]
    res = run_bass_kernel_spmd(nc, in_maps, core_ids=list(range(N_CORES)))
    return np.concatenate([res.results[c]["out"] for c in range(N_CORES)], axis=0)

